# revision 5
# baseline (speedup 1.0000x reference)
"""Trainium2 Bass kernel for nn_KnowledgeSelector (sparse additive attention).

Sharding: pure data-parallel — batch B=8 across 8 NeuronCores, weights
replicated. Per core the dominant work is the additive-attention score
matrix m[i,j] = v . tanh(wq[i,:] + uh[j,:]) (33.5M tanh evals -> ACT bound).

Device-side decomposition per core (batch b), all activations kept in
[feature(partition), token(free)] layout, fp16 matmuls w/ fp32 PSUM:
  1. highway(b|c): 2 layers of sigmoid/relu gated linear maps.
  2. wq^T = Wq^T hb^T ; uh^T = Wk^T hc^T (+bq) ; area: uha^T, wqa.
  3. phase-2: per h-chunk (4) / j-group (16 j): DVE broadcast-add
     (wq^T + uh_j as per-partition scalar), one big ACT Tanh over the
     group, then per-j matmul with a shifted-window stationary (v placed
     in column j, zeros elsewhere) accumulating the full m matrix
     [128 j, 512 i] into ONE PSUM bank.  b_score = masked row max via
     PE transpose + DVE free-dim max.
  4. windows: all 402 multi-scale window sums (softmax numerators/denoms,
     b_score window sums) as ONE banded 0/1 matmul vs [E*b_enc | E | bs];
     softmax over 402 on a single row; final out = r^T seg_unnorm.
"""

import numpy as np

B, LB, LC, H = 8, 512, 128, 256
H2 = 2 * H
MIN_WS, N_WIN = 5, 4
N_SEG = [(LB - MIN_WS * (i + 1)) // MIN_WS + 1 for i in range(N_WIN)]
N_TOT = sum(N_SEG)  # 402
JG = 16             # j's per phase-2 ACT group
N_CORES = 8

_cache: dict = {}


def _build_window_matrix_T() -> np.ndarray:
    """Bwin^T padded to [512, 512] fp16 (cols >= N_TOT are zero)."""
    Bw = np.zeros((512, LB), np.float16)
    r = 0
    for i in range(N_WIN):
        ws = MIN_WS * (i + 1)
        for n in range(N_SEG[i]):
            Bw[r, n * MIN_WS: n * MIN_WS + ws] = 1.0
            r += 1
    return np.ascontiguousarray(Bw.T)  # [512 t, 512 win]


def _build_program():
    if "prog" in _cache:
        return _cache["prog"]
    import concourse.bass as bass  # noqa: F401
    import concourse.tile as tile
    from concourse import bacc, mybir

    f16 = mybir.dt.float16
    f32 = mybir.dt.float32
    AF = mybir.ActivationFunctionType
    AX = mybir.AxisListType

    nc = bacc.Bacc("TRN2", target_bir_lowering=False, debug=False,
                   num_devices=N_CORES)

    def din(name, shape, dt=f16):
        return nc.dram_tensor(name, shape, dt, kind="ExternalInput").ap()

    # ---- DRAM inputs (host-prepped layouts) ----
    d_bT = din("bT", [2, 128, 512])       # b_enc^T   [hchunk, p, t]
    d_bE = din("bE", [4, 128, 256])       # b_enc     [ttile, p, h]
    d_cT = din("cT", [2, 128, 128])       # c_enc^T
    d_csb = din("csb", [2, 128, 512])     # c_state^T broadcast along t
    d_csT = din("csT", [2, 128, 1])       # c_state^T
    d_Wb = din("Wb", [24, 128, 512])      # [(l*3+m)*4+kt, p, mcol] m: 0=g,1=n,2=l
    d_Wc = din("Wc", [24, 128, 512])
    d_biasb = din("biasb", [128, 24], f32)  # col=(l*3+m)*4+mc
    d_biasc = din("biasc", [128, 24], f32)
    d_mWq = din("mWq", [4, 128, 512])
    d_mWk = din("mWk", [4, 128, 512])
    d_mbq = din("mbq", [128, 4], f32)
    d_aWq = din("aWq", [2, 128, 256])
    d_aWk = din("aWk", [2, 128, 256])
    d_abq = din("abq", [128, 2], f32)
    d_va = din("va", [128, 2])
    d_vz = din("vz", [128, 1020])         # vz[p, c*255+127]=v[c*128+p], else 0
    d_BT = din("BT", [4, 128, 512])       # window matrix^T  [ttile, p, win]
    d_eye = din("eye", [128, 128], f32)
    d_cmadd = din("cmadd", [128, 1], f32)  # (c_mask-1)*1e9
    d_bmask = din("bmask", [128, 4], f32)  # b_mask [p, ttile]

    o_out = nc.dram_tensor("o_out", [1, 256], f32, kind="ExternalOutput").ap()
    o_ss = nc.dram_tensor("o_ss", [1, N_TOT], f32, kind="ExternalOutput").ap()
    o_bs = nc.dram_tensor("o_bs", [128, 4], f32, kind="ExternalOutput").ap()

    with tile.TileContext(nc) as tc:
        from contextlib import ExitStack
        ctx = ExitStack()
        with ctx:
            cpool = ctx.enter_context(tc.tile_pool(name="consts", bufs=1))
            apool = ctx.enter_context(tc.tile_pool(name="acts", bufs=1))

            def cload(name, dram, n, inner, dt):
                """n slices of [128, inner] -> sbuf tile [128, n, inner]."""
                if n is None:
                    t = cpool.tile(inner, dt, tag=name, name=name)
                    nc.sync.dma_start(t[:], dram[:])
                    return t
                t = cpool.tile([128, n, inner], dt, tag=name, name=name)
                for i in range(n):
                    nc.sync.dma_start(t[:, i, :], dram[i])
                return t

            # ---- load everything (weights fully resident in SBUF) ----
            bT = cload("bT", d_bT, 2, 512, f16)
            Wb = cload("Wb", d_Wb, 24, 512, f16)
            biasb = cload("biasb", d_biasb, None, [128, 24], f32)
            csb = cload("csb", d_csb, 2, 512, f16)
            cT = cload("cT", d_cT, 2, 128, f16)
            Wc = cload("Wc", d_Wc, 24, 512, f16)
            biasc = cload("biasc", d_biasc, None, [128, 24], f32)
            csT = cload("csT", d_csT, 2, 1, f16)
            mWq = cload("mWq", d_mWq, 4, 512, f16)
            mWk = cload("mWk", d_mWk, 4, 512, f16)
            mbq = cload("mbq", d_mbq, None, [128, 4], f32)
            aWq = cload("aWq", d_aWq, 2, 256, f16)
            aWk = cload("aWk", d_aWk, 2, 256, f16)
            abq = cload("abq", d_abq, None, [128, 2], f32)
            va = cload("va", d_va, None, [128, 2], f16)
            vz = cload("vz", d_vz, None, [128, 1020], f16)
            eye = cload("eye", d_eye, None, [128, 128], f32)
            cmadd = cload("cmadd", d_cmadd, None, [128, 1], f32)
            bmask = cload("bmask", d_bmask, None, [128, 4], f32)
            bE = cload("bE", d_bE, 4, 256, f16)
            BTw = cload("BT", d_BT, 4, 512, f16)

            # ---- highway (b side T=512, c side T=128) ----
            def highway(xin, W, biases, T, side):
                """xin: list of 4 APs [128, T]; returns list of 4 tiles."""
                cur = xin
                with tc.tile_pool(name=f"ps_hw{side}", bufs=2,
                                  space="PSUM") as pp:
                    for l in range(2):
                        nxt = []
                        for mc in range(4):
                            ps = {}
                            for m, pname in ((0, "pg"), (1, "pn"), (2, "pl")):
                                pt = pp.tile([128, T], f32, tag=pname, name=pname)
                                for kt in range(4):
                                    nc.tensor.matmul(
                                        pt[:],
                                        W[:, (l * 3 + m) * 4 + kt,
                                          mc * 128:(mc + 1) * 128],
                                        cur[kt][:],
                                        start=(kt == 0), stop=(kt == 3))
                                ps[pname] = pt
                            def bc(m):
                                i = (l * 3 + m) * 4 + mc
                                return biases[:, i:i + 1]
                            g = apool.tile([128, T], f16, tag=f"g_{side}", bufs=2, name="g")
                            r = apool.tile([128, T], f16, tag=f"r_{side}", bufs=2, name="r")
                            lc = apool.tile([128, T], f16, tag=f"lc_{side}", bufs=2, name="lc")
                            nc.scalar.activation(g[:], ps["pg"][:], AF.Sigmoid, bias=bc(0))
                            nc.scalar.activation(r[:], ps["pn"][:], AF.Relu, bias=bc(1))
                            nc.vector.tensor_scalar_add(lc[:], ps["pl"][:], bc(2))
                            d = apool.tile([128, T], f16, tag=f"d_{side}", bufs=2, name="d")
                            nc.vector.tensor_sub(d[:], r[:], lc[:])
                            gd = apool.tile([128, T], f16, tag=f"gd_{side}", bufs=2, name="gd")
                            nc.vector.tensor_mul(gd[:], g[:], d[:])
                            o = apool.tile([128, T], f16, tag=f"hw_{side}{l}{mc}", name="o")
                            nc.vector.tensor_add(o[:], gd[:], lc[:])
                            nxt.append(o)
                        cur = nxt
                return cur

            xb = [bT[:, 0, :], bT[:, 1, :], csb[:, 0, :], csb[:, 1, :]]
            hb = highway(xb, Wb, biasb, 512, "b")
            xc = [cT[:, 0, :], cT[:, 1, :], csb[:, 0, 0:128], csb[:, 1, 0:128]]
            hc = highway(xc, Wc, biasc, 128, "c")

            # ---- projections: wq^T [128,(4),512], uh^T [128,(4),128] ----
            wq = apool.tile([128, 4, 512], f16, tag="wq", name="wq")
            uh = apool.tile([128, 4, 128], f32, tag="uh", name="uh")
            with tc.tile_pool(name="ps_proj", bufs=2, space="PSUM") as pp:
                for mc in range(4):
                    pq = pp.tile([128, 512], f32, tag="pq", name="pq")
                    for kt in range(4):
                        nc.tensor.matmul(pq[:], mWq[:, kt, mc * 128:(mc + 1) * 128],
                                         hb[kt][:], start=(kt == 0), stop=(kt == 3))
                    nc.vector.tensor_copy(wq[:, mc, :], pq[:])
                for mc in range(4):
                    pu = pp.tile([128, 128], f32, tag="pu", name="pu")
                    for kt in range(4):
                        nc.tensor.matmul(pu[:], mWk[:, kt, mc * 128:(mc + 1) * 128],
                                         hc[kt][:], start=(kt == 0), stop=(kt == 3))
                    nc.vector.tensor_scalar_add(uh[:, mc, :], pu[:],
                                                mbq[:, mc:mc + 1])

            # ---- area attention scores s_a[t] -> E = exp(s_a) [128, 4] ----
            E = apool.tile([128, 4], f32, tag="E", name="E")
            sa_row = apool.tile([1, 512], f32, tag="sa_row", name="sa_row")
            with tc.tile_pool(name="ps_area", bufs=2, space="PSUM") as pp:
                wqa = apool.tile([128, 2], f32, tag="wqa", name="wqa")
                for c in range(2):
                    pwq = pp.tile([128, 1], f32, tag="pwqa", name="pwq")
                    for kt in range(2):
                        nc.tensor.matmul(pwq[:], aWq[:, kt, c * 128:(c + 1) * 128],
                                         csT[:, kt, :], start=(kt == 0), stop=(kt == 1))
                    nc.vector.tensor_scalar_add(wqa[:, c:c + 1], pwq[:],
                                                abq[:, c:c + 1])
                sat = []
                for c in range(2):
                    pa = pp.tile([128, 512], f32, tag="pa", name="pa")
                    for kt in range(2):
                        nc.tensor.matmul(pa[:], aWk[:, kt, c * 128:(c + 1) * 128],
                                         bT[:, kt, :], start=(kt == 0), stop=(kt == 1))
                    st = apool.tile([128, 512], f16, tag=f"sat{c}", name="st")
                    nc.scalar.activation(st[:], pa[:], AF.Tanh, bias=wqa[:, c:c + 1])
                    sat.append(st)
                psa = pp.tile([1, 512], f32, tag="psa", name="psa")
                for c in range(2):
                    nc.tensor.matmul(psa[:], va[:, c:c + 1], sat[c][:],
                                     start=(c == 0), stop=(c == 1))
                nc.vector.tensor_copy(sa_row[:], psa[:])
            with tc.tile_pool(name="ps_e", bufs=2, space="PSUM") as pp:
                for c in range(4):
                    pte = pp.tile([128, 1], f32, tag="pte", name="pte")
                    nc.tensor.transpose(pte[:], sa_row[0:1, c * 128:(c + 1) * 128],
                                        eye[0:1, 0:1])
                    nc.scalar.activation(E[:, c:c + 1], pte[:], AF.Exp)

            # ---- PHASE 2: m[j, i] accumulation in one PSUM bank ----
            bs_raw = apool.tile([128, 4], f32, tag="bs_raw", name="bs_raw")
            with tc.tile_pool(name="ps_m", bufs=1, space="PSUM") as pm, \
                 tc.tile_pool(name="stgp", bufs=2) as stgp, \
                 tc.tile_pool(name="tnhp", bufs=2) as tnhp, \
                 tc.tile_pool(name="ps_t", bufs=2, space="PSUM") as pst:
                psum_m = pm.tile([128, 512], f32, tag="m", name="psum_m")
                NG = 128 // JG
                for ht in range(4):
                    for g in range(NG):
                        stg = stgp.tile([128, JG, 512], f16, tag="stg", name="stg")
                        for j8 in range(JG):
                            j = g * JG + j8
                            nc.vector.tensor_scalar_add(
                                stg[:, j8, :], wq[:, ht, :], uh[:, ht, j:j + 1])
                        tnh = tnhp.tile([128, JG, 512], f16, tag="tnh", name="tnh")
                        nc.scalar.activation(tnh[:], stg[:], AF.Tanh)
                        for j8 in range(JG):
                            j = g * JG + j8
                            off = ht * 255 + 127 - j
                            nc.tensor.matmul(
                                psum_m[:], vz[:, off:off + 128], tnh[:, j8, :],
                                start=(ht == 0 and j == 0),
                                stop=(ht == 3 and j == 127))
                # ---- b_score: masked max over j ----
                for ib in range(4):
                    msb = apool.tile([128, 128], f32, tag="msb", bufs=2, name="msb")
                    nc.vector.tensor_scalar_add(
                        msb[:], psum_m[:, ib * 128:(ib + 1) * 128], cmadd[:])
                    pT = pst.tile([128, 128], f32, tag="pT", name="pT")
                    nc.tensor.transpose(pT[:], msb[:], eye[:])
                    nc.vector.reduce_max(bs_raw[:, ib:ib + 1], pT[:], axis=AX.X)

            bs = apool.tile([128, 4], f32, tag="bs", name="bs")
            nc.vector.tensor_mul(bs[:], bs_raw[:], bmask[:])
            bs16 = apool.tile([128, 4], f16, tag="bs16", name="bs16")
            nc.vector.tensor_copy(bs16[:], bs[:])
            nc.sync.dma_start(o_bs[:], bs[:])

            # ---- windows: [Z | E | bs] through banded matmul ----
            sc_row = apool.tile([1, 512], f32, tag="sc_row", name="sc_row")
            ss_row = apool.tile([1, 512], f32, tag="ss_row", name="ss_row")
            den = apool.tile([128, 4], f32, tag="den", name="den")
            rden = apool.tile([128, 4], f32, tag="rden", name="rden")
            r16 = apool.tile([128, 4], f16, tag="r16", name="r16")
            out_sb = apool.tile([1, 256], f32, tag="out_sb", name="out_sb")
            with tc.tile_pool(name="ps_w", bufs=4, space="PSUM") as pw_pool, \
                 tc.tile_pool(name="ps_t2", bufs=1, space="PSUM") as pst2, \
                 tc.tile_pool(name="ps_o", bufs=1, space="PSUM") as po_pool:
                zt = []
                for kt in range(4):
                    z = apool.tile([128, 258], f16, tag=f"zt{kt}", name="z")
                    nc.vector.tensor_scalar_mul(z[:, 0:256], bE[:, kt, :],
                                                E[:, kt:kt + 1])
                    nc.vector.tensor_copy(z[:, 256:257], E[:, kt:kt + 1])
                    nc.vector.tensor_copy(z[:, 257:258], bs16[:, kt:kt + 1])
                    zt.append(z)
                pw = []
                for mc in range(4):
                    p = pw_pool.tile([128, 258], f32, tag="pw", name="p")
                    for kt in range(4):
                        nc.tensor.matmul(p[:], BTw[:, kt, mc * 128:(mc + 1) * 128],
                                         zt[kt][:], start=(kt == 0), stop=(kt == 3))
                    pw.append(p)
                # win score row -> [1, 512]
                for mc in range(4):
                    wcol = apool.tile([128, 1], f32, tag="wcol", bufs=2, name="wcol")
                    nc.vector.tensor_copy(wcol[:], pw[mc][:, 257:258])
                    prT = pst2.tile([1, 128], f32, tag="prT", name="prT")
                    nc.tensor.transpose(prT[:], wcol[:], eye[:])
                    nc.vector.tensor_copy(sc_row[0:1, mc * 128:(mc + 1) * 128], prT[:])
                nc.vector.memset(sc_row[0:1, N_TOT:512], -1e9)
                # softmax over the 402 window scores
                mx = apool.tile([1, 1], f32, tag="mx", name="mx")
                nc.vector.reduce_max(mx[:], sc_row[:], axis=AX.X)
                nmx = apool.tile([1, 1], f32, tag="nmx", name="nmx")
                nc.vector.tensor_scalar_mul(nmx[:], mx[:], -1.0)
                es_row = apool.tile([1, 512], f32, tag="es_row", name="es_row")
                nc.scalar.activation(es_row[:], sc_row[:], AF.Exp, bias=nmx[0:1, 0:1])
                sm = apool.tile([1, 1], f32, tag="sm", name="sm")
                nc.vector.reduce_sum(sm[:], es_row[:], axis=AX.X)
                rsm = apool.tile([1, 1], f32, tag="rsm", name="rsm")
                nc.vector.reciprocal(rsm[:], sm[:])
                nc.vector.tensor_scalar_mul(ss_row[:], es_row[:], rsm[0:1, 0:1])
                nc.sync.dma_start(o_ss[:], ss_row[0:1, 0:N_TOT])
                # r = s_score / denom  (per window, as columns)
                for mc in range(4):
                    nc.vector.tensor_scalar_add(den[:, mc:mc + 1],
                                                pw[mc][:, 256:257], 1e-30)
                    nc.vector.reciprocal(rden[:, mc:mc + 1], den[:, mc:mc + 1])
                    pcol = pst2.tile([128, 1], f32, tag="pcol", name="pcol")
                    nc.tensor.transpose(pcol[:], ss_row[0:1, mc * 128:(mc + 1) * 128],
                                        eye[0:1, 0:1])
                    nc.vector.tensor_mul(r16[:, mc:mc + 1], pcol[:],
                                         rden[:, mc:mc + 1])
                # out = sum_n r[n] * seg_unnorm[n, :]
                pout = po_pool.tile([1, 256], f32, tag="pout", name="pout")
                for mc in range(4):
                    seg = apool.tile([128, 256], f16, tag="seg", bufs=2, name="seg")
                    nc.vector.tensor_copy(seg[:], pw[mc][:, 0:256])
                    nc.tensor.matmul(pout[:], r16[:, mc:mc + 1], seg[:],
                                     start=(mc == 0), stop=(mc == 3))
                nc.vector.tensor_copy(out_sb[:], pout[:])
                nc.sync.dma_start(o_out[:], out_sb[:])

    nc.compile()
    _cache["prog"] = nc
    return nc


def _prep_shared(inputs):
    """Host-side packing of replicated (weight) tensors."""
    f16 = np.float16
    f32 = np.float32
    sh = {}
    # Wb / Wc: [(l*3+m)*4+kt, 128, 512]; m order: 0=Wg, 1=Wn, 2=Wl
    for side, pfx in (("b", "bh"), ("c", "ch")):
        mats = [inputs[f"{pfx}_Wg"], inputs[f"{pfx}_Wn"], inputs[f"{pfx}_Wl"]]
        tiles = np.empty((24, 128, 512), f16)
        for l in range(2):
            for m in range(3):
                Wf = np.asarray(mats[m][l], f32).astype(f16)  # [512, 512]
                tiles[(l * 3 + m) * 4:(l * 3 + m) * 4 + 4] = Wf.reshape(4, 128, 512)
        sh[f"W{side}"] = tiles
        bias = np.empty((128, 24), f32)
        bmats = [inputs[f"{pfx}_bg"], inputs[f"{pfx}_bn"], inputs[f"{pfx}_bl"]]
        for l in range(2):
            for m in range(3):
                bf = np.asarray(bmats[m][l], f32)  # [512]
                bias[:, (l * 3 + m) * 4:(l * 3 + m) * 4 + 4] = bf.reshape(4, 128).T
        sh[f"bias{side}"] = bias
    sh["mWq"] = np.asarray(inputs["m_Wq"], f32).astype(f16).reshape(4, 128, 512)
    sh["mWk"] = np.asarray(inputs["m_Wk"], f32).astype(f16).reshape(4, 128, 512)
    sh["mbq"] = np.ascontiguousarray(
        np.asarray(inputs["m_bq"], f32).reshape(4, 128).T)
    sh["aWq"] = np.asarray(inputs["a_Wq"], f32).astype(f16).reshape(2, 128, 256)
    sh["aWk"] = np.asarray(inputs["a_Wk"], f32).astype(f16).reshape(2, 128, 256)
    sh["abq"] = np.ascontiguousarray(
        np.asarray(inputs["a_bq"], f32).reshape(2, 128).T)
    sh["va"] = np.ascontiguousarray(
        np.asarray(inputs["a_v"], f32).astype(f16).reshape(2, 128).T)
    vz = np.zeros((128, 1020), f16)
    mv = np.asarray(inputs["m_v"], f32).astype(f16)
    for c in range(4):
        vz[:, c * 255 + 127] = mv[c * 128:(c + 1) * 128]
    sh["vz"] = vz
    sh["BT"] = np.ascontiguousarray(
        _build_window_matrix_T().reshape(4, 128, 512))
    sh["eye"] = np.eye(128, dtype=f32)
    return sh


def kernel(**inputs):
    nc = _build_program()
    from concourse import bass_utils

    f16 = np.float16
    f32 = np.float32
    sh = _prep_shared(inputs)

    b_enc = np.asarray(inputs["b_enc_output"], f32)
    c_enc = np.asarray(inputs["c_enc_output"], f32)
    c_state = np.asarray(inputs["c_state"], f32)
    b_mask = np.asarray(inputs["b_mask"]).astype(f32)
    c_mask = np.asarray(inputs["c_mask"]).astype(f32)

    in_maps = []
    for b in range(N_CORES):
        be = b_enc[b].astype(f16)                      # [512, 256]
        beT = np.ascontiguousarray(be.T)               # [256, 512]
        ce = c_enc[b].astype(f16)
        csv = c_state[b].astype(f16)                   # [1, 256]
        csTv = np.ascontiguousarray(csv.T)             # [256, 1]
        m = {
            "bT": beT.reshape(2, 128, 512),
            "bE": be.reshape(4, 128, 256),
            "cT": np.ascontiguousarray(ce.T).reshape(2, 128, 128),
            "csb": np.ascontiguousarray(
                np.broadcast_to(csTv, (256, 512))).reshape(2, 128, 512),
            "csT": csTv.reshape(2, 128, 1),
            "Wb": sh["Wb"], "Wc": sh["Wc"],
            "biasb": sh["biasb"], "biasc": sh["biasc"],
            "mWq": sh["mWq"], "mWk": sh["mWk"], "mbq": sh["mbq"],
            "aWq": sh["aWq"], "aWk": sh["aWk"], "abq": sh["abq"],
            "va": sh["va"], "vz": sh["vz"], "BT": sh["BT"], "eye": sh["eye"],
            "cmadd": ((c_mask[b] - 1.0) * 1e9).reshape(128, 1).astype(f32),
            "bmask": np.ascontiguousarray(
                b_mask[b].reshape(4, 128).T).astype(f32),
        }
        in_maps.append(m)

    res = bass_utils.run_bass_kernel_spmd(nc, in_maps,
                                          core_ids=list(range(N_CORES)))
    out = np.stack([res.results[c]["o_out"] for c in range(N_CORES)])  # [8,1,256]
    ss = np.stack([res.results[c]["o_ss"][0] for c in range(N_CORES)])  # [8,402]
    bs = np.stack([np.ascontiguousarray(res.results[c]["o_bs"].T).reshape(512)
                   for c in range(N_CORES)])                            # [8,512]
    return (out.astype(f32), ss.astype(f32), bs.astype(f32))


# revision 7
# speedup vs baseline: 1.1321x; 1.1321x over previous
"""Trainium2 Bass kernel for nn_KnowledgeSelector (sparse additive attention).

Sharding: pure data-parallel — batch B=8 across 8 NeuronCores, weights
replicated. Per core the dominant work is the additive-attention score
matrix m[i,j] = v . tanh(wq[i,:] + uh[j,:]) (33.5M tanh evals -> ACT bound).

Device-side decomposition per core (batch b), all activations kept in
[feature(partition), token(free)] layout, fp16 matmuls w/ fp32 PSUM:
  1. highway(b|c): 2 layers of sigmoid/relu gated linear maps.
  2. wq^T = Wq^T hb^T ; uh^T = Wk^T hc^T (+bq) ; area: uha^T, wqa.
  3. phase-2: per h-chunk (4) / j-group (16 j): DVE broadcast-add
     (wq^T + uh_j as per-partition scalar), one big ACT Tanh over the
     group, then per-j matmul with a shifted-window stationary (v placed
     in column j, zeros elsewhere) accumulating the full m matrix
     [128 j, 512 i] into ONE PSUM bank.  b_score = masked row max via
     PE transpose + DVE free-dim max.
  4. windows: all 402 multi-scale window sums (softmax numerators/denoms,
     b_score window sums) as ONE banded 0/1 matmul vs [E*b_enc | E | bs];
     softmax over 402 on a single row; final out = r^T seg_unnorm.
"""

import numpy as np

B, LB, LC, H = 8, 512, 128, 256
H2 = 2 * H
MIN_WS, N_WIN = 5, 4
N_SEG = [(LB - MIN_WS * (i + 1)) // MIN_WS + 1 for i in range(N_WIN)]
N_TOT = sum(N_SEG)  # 402
JG = 8              # j's per phase-2 ACT group
N_CORES = 8

_cache: dict = {}


def _build_window_matrix_T() -> np.ndarray:
    """Bwin^T padded to [512, 512] fp16 (cols >= N_TOT are zero)."""
    Bw = np.zeros((512, LB), np.float16)
    r = 0
    for i in range(N_WIN):
        ws = MIN_WS * (i + 1)
        for n in range(N_SEG[i]):
            Bw[r, n * MIN_WS: n * MIN_WS + ws] = 1.0
            r += 1
    return np.ascontiguousarray(Bw.T)  # [512 t, 512 win]


def _build_program():
    if "prog" in _cache:
        return _cache["prog"]
    import concourse.bass as bass  # noqa: F401
    import concourse.tile as tile
    from concourse import bacc, mybir

    f16 = mybir.dt.float16
    f32 = mybir.dt.float32
    AF = mybir.ActivationFunctionType
    AX = mybir.AxisListType

    nc = bacc.Bacc("TRN2", target_bir_lowering=False, debug=False,
                   num_devices=N_CORES)

    def din(name, shape, dt=f16):
        return nc.dram_tensor(name, shape, dt, kind="ExternalInput").ap()

    # ---- DRAM inputs (host-prepped layouts) ----
    d_bT = din("bT", [2, 128, 512])       # b_enc^T   [hchunk, p, t]
    d_bE = din("bE", [4, 128, 256])       # b_enc     [ttile, p, h]
    d_cT = din("cT", [2, 128, 128])       # c_enc^T
    d_csb = din("csb", [2, 128, 512])     # c_state^T broadcast along t
    d_csT = din("csT", [2, 128, 1])       # c_state^T
    d_Wb = din("Wb", [24, 128, 512])      # [(l*3+m)*4+kt, p, mcol] m: 0=g,1=n,2=l
    d_Wc = din("Wc", [24, 128, 512])
    d_biasb = din("biasb", [128, 24], f32)  # col=(l*3+m)*4+mc
    d_biasc = din("biasc", [128, 24], f32)
    d_mWq = din("mWq", [4, 128, 512])
    d_mWk = din("mWk", [4, 128, 512])
    d_mbq = din("mbq", [128, 4], f32)
    d_aWq = din("aWq", [2, 128, 256])
    d_aWk = din("aWk", [2, 128, 256])
    d_abq = din("abq", [128, 2], f32)
    d_va = din("va", [128, 2])
    d_vz = din("vz", [128, 1020])         # vz[p, c*255+127]=v[c*128+p], else 0
    d_BT = din("BT", [4, 128, 512])       # window matrix^T  [ttile, p, win]
    d_eye = din("eye", [128, 128], f32)
    d_cmadd = din("cmadd", [128, 1], f32)  # (c_mask-1)*1e9
    d_bmask = din("bmask", [128, 4], f32)  # b_mask [p, ttile]

    o_out = nc.dram_tensor("o_out", [1, 256], f32, kind="ExternalOutput").ap()
    o_ss = nc.dram_tensor("o_ss", [1, N_TOT], f32, kind="ExternalOutput").ap()
    o_bs = nc.dram_tensor("o_bs", [128, 4], f32, kind="ExternalOutput").ap()

    with tile.TileContext(nc) as tc:
        from contextlib import ExitStack
        ctx = ExitStack()
        with ctx:
            cpool = ctx.enter_context(tc.tile_pool(name="consts", bufs=1))
            apool = ctx.enter_context(tc.tile_pool(name="acts", bufs=1))

            def cload(name, dram, n, inner, dt):
                """n slices of [128, inner] -> sbuf tile [128, n, inner]."""
                if n is None:
                    t = cpool.tile(inner, dt, tag=name, name=name)
                    nc.sync.dma_start(t[:], dram[:])
                    return t
                t = cpool.tile([128, n, inner], dt, tag=name, name=name)
                for i in range(n):
                    nc.sync.dma_start(t[:, i, :], dram[i])
                return t

            # ---- load everything (weights fully resident in SBUF) ----
            bT = cload("bT", d_bT, 2, 512, f16)
            Wb = cload("Wb", d_Wb, 24, 512, f16)
            biasb = cload("biasb", d_biasb, None, [128, 24], f32)
            csb = cload("csb", d_csb, 2, 512, f16)
            cT = cload("cT", d_cT, 2, 128, f16)
            Wc = cload("Wc", d_Wc, 24, 512, f16)
            biasc = cload("biasc", d_biasc, None, [128, 24], f32)
            csT = cload("csT", d_csT, 2, 1, f16)
            mWq = cload("mWq", d_mWq, 4, 512, f16)
            mWk = cload("mWk", d_mWk, 4, 512, f16)
            mbq = cload("mbq", d_mbq, None, [128, 4], f32)
            aWq = cload("aWq", d_aWq, 2, 256, f16)
            aWk = cload("aWk", d_aWk, 2, 256, f16)
            abq = cload("abq", d_abq, None, [128, 2], f32)
            va = cload("va", d_va, None, [128, 2], f16)
            vz = cload("vz", d_vz, None, [128, 1020], f16)
            eye = cload("eye", d_eye, None, [128, 128], f32)
            cmadd = cload("cmadd", d_cmadd, None, [128, 1], f32)
            bmask = cload("bmask", d_bmask, None, [128, 4], f32)
            bE = cload("bE", d_bE, 4, 256, f16)
            BTw = cload("BT", d_BT, 4, 512, f16)

            # ---- highway (b side T=512, c side T=128) ----
            def highway(xin, W, biases, T, side):
                """xin: list of 4 APs [128, T]; returns list of 4 tiles."""
                cur = xin
                with tc.tile_pool(name=f"ps_hw{side}", bufs=2,
                                  space="PSUM") as pp:
                    for l in range(2):
                        nxt = []
                        for mc in range(4):
                            ps = {}
                            for m, pname in ((0, "pg"), (1, "pn"), (2, "pl")):
                                pt = pp.tile([128, T], f32, tag=pname, name=pname)
                                for kt in range(4):
                                    nc.tensor.matmul(
                                        pt[:],
                                        W[:, (l * 3 + m) * 4 + kt,
                                          mc * 128:(mc + 1) * 128],
                                        cur[kt][:],
                                        start=(kt == 0), stop=(kt == 3))
                                ps[pname] = pt
                            def bc(m):
                                i = (l * 3 + m) * 4 + mc
                                return biases[:, i:i + 1]
                            g = apool.tile([128, T], f16, tag=f"g_{side}", bufs=2, name="g")
                            r = apool.tile([128, T], f16, tag=f"r_{side}", bufs=2, name="r")
                            lc = apool.tile([128, T], f16, tag=f"lc_{side}", bufs=2, name="lc")
                            nc.scalar.activation(g[:], ps["pg"][:], AF.Sigmoid, bias=bc(0))
                            nc.scalar.activation(r[:], ps["pn"][:], AF.Relu, bias=bc(1))
                            nc.vector.tensor_scalar_add(lc[:], ps["pl"][:], bc(2))
                            d = apool.tile([128, T], f16, tag=f"d_{side}", bufs=2, name="d")
                            nc.vector.tensor_sub(d[:], r[:], lc[:])
                            gd = apool.tile([128, T], f16, tag=f"gd_{side}", bufs=2, name="gd")
                            nc.vector.tensor_mul(gd[:], g[:], d[:])
                            o = apool.tile([128, T], f16, tag=f"hw_{side}{l}{mc}", name="o")
                            nc.vector.tensor_add(o[:], gd[:], lc[:])
                            nxt.append(o)
                        cur = nxt
                return cur

            xb = [bT[:, 0, :], bT[:, 1, :], csb[:, 0, :], csb[:, 1, :]]
            hb = highway(xb, Wb, biasb, 512, "b")
            xc = [cT[:, 0, :], cT[:, 1, :], csb[:, 0, 0:128], csb[:, 1, 0:128]]
            hc = highway(xc, Wc, biasc, 128, "c")

            # ---- projections: wq^T [128,(4),512], uh^T [128,(4),128] ----
            wq = apool.tile([128, 4, 512], f16, tag="wq", name="wq")
            uh = apool.tile([128, 4, 128], f32, tag="uh", name="uh")
            with tc.tile_pool(name="ps_proj", bufs=2, space="PSUM") as pp:
                for mc in range(4):
                    pq = pp.tile([128, 512], f32, tag="pq", name="pq")
                    for kt in range(4):
                        nc.tensor.matmul(pq[:], mWq[:, kt, mc * 128:(mc + 1) * 128],
                                         hb[kt][:], start=(kt == 0), stop=(kt == 3))
                    nc.vector.tensor_copy(wq[:, mc, :], pq[:])
                for mc in range(4):
                    pu = pp.tile([128, 128], f32, tag="pu", name="pu")
                    for kt in range(4):
                        nc.tensor.matmul(pu[:], mWk[:, kt, mc * 128:(mc + 1) * 128],
                                         hc[kt][:], start=(kt == 0), stop=(kt == 3))
                    nc.vector.tensor_scalar_add(uh[:, mc, :], pu[:],
                                                mbq[:, mc:mc + 1])

            # ---- area attention scores s_a[t] -> E = exp(s_a) [128, 4] ----
            E = apool.tile([128, 4], f32, tag="E", name="E")
            sa_row = apool.tile([1, 512], f32, tag="sa_row", name="sa_row")
            with tc.tile_pool(name="ps_area", bufs=2, space="PSUM") as pp:
                wqa = apool.tile([128, 2], f32, tag="wqa", name="wqa")
                for c in range(2):
                    pwq = pp.tile([128, 1], f32, tag="pwqa", name="pwq")
                    for kt in range(2):
                        nc.tensor.matmul(pwq[:], aWq[:, kt, c * 128:(c + 1) * 128],
                                         csT[:, kt, :], start=(kt == 0), stop=(kt == 1))
                    nc.vector.tensor_scalar_add(wqa[:, c:c + 1], pwq[:],
                                                abq[:, c:c + 1])
                sat = []
                for c in range(2):
                    pa = pp.tile([128, 512], f32, tag="pa", name="pa")
                    for kt in range(2):
                        nc.tensor.matmul(pa[:], aWk[:, kt, c * 128:(c + 1) * 128],
                                         bT[:, kt, :], start=(kt == 0), stop=(kt == 1))
                    st = apool.tile([128, 512], f16, tag=f"sat{c}", name="st")
                    nc.scalar.activation(st[:], pa[:], AF.Tanh, bias=wqa[:, c:c + 1])
                    sat.append(st)
                psa = pp.tile([1, 512], f32, tag="psa", name="psa")
                for c in range(2):
                    nc.tensor.matmul(psa[:], va[:, c:c + 1], sat[c][:],
                                     start=(c == 0), stop=(c == 1))
                nc.vector.tensor_copy(sa_row[:], psa[:])
            with tc.tile_pool(name="ps_e", bufs=2, space="PSUM") as pp:
                for c in range(4):
                    pte = pp.tile([128, 1], f32, tag="pte", name="pte")
                    nc.tensor.transpose(pte[:], sa_row[0:1, c * 128:(c + 1) * 128],
                                        eye[0:1, 0:1])
                    nc.scalar.activation(E[:, c:c + 1], pte[:], AF.Exp)

            # ---- PHASE 2: m[j, i] accumulation in one PSUM bank ----
            bs_raw = apool.tile([128, 4], f32, tag="bs_raw", name="bs_raw")
            with tc.tile_pool(name="ps_m", bufs=1, space="PSUM") as pm, \
                 tc.tile_pool(name="stgp", bufs=3) as stgp, \
                 tc.tile_pool(name="tnhp", bufs=3) as tnhp, \
                 tc.tile_pool(name="ps_t", bufs=2, space="PSUM") as pst:
                psum_m = pm.tile([128, 512], f32, tag="m", name="psum_m")
                NG = 128 // JG
                for ht in range(4):
                    for g in range(NG):
                        stg = stgp.tile([128, JG, 512], f16, tag="stg", name="stg")
                        for j8 in range(JG):
                            j = g * JG + j8
                            nc.vector.tensor_scalar_add(
                                stg[:, j8, :], wq[:, ht, :], uh[:, ht, j:j + 1])
                        tnh = tnhp.tile([128, JG, 512], f16, tag="tnh", name="tnh")
                        nc.scalar.activation(tnh[:], stg[:], AF.Tanh)
                        for j8 in range(JG):
                            j = g * JG + j8
                            off = ht * 255 + 127 - j
                            nc.tensor.matmul(
                                psum_m[:], vz[:, off:off + 128], tnh[:, j8, :],
                                start=(ht == 0 and j == 0),
                                stop=(ht == 3 and j == 127))
                # ---- b_score: masked max over j ----
                for ib in range(4):
                    msb = apool.tile([128, 128], f32, tag="msb", bufs=2, name="msb")
                    nc.vector.tensor_scalar_add(
                        msb[:], psum_m[:, ib * 128:(ib + 1) * 128], cmadd[:])
                    pT = pst.tile([128, 128], f32, tag="pT", name="pT")
                    nc.tensor.transpose(pT[:], msb[:], eye[:])
                    nc.vector.reduce_max(bs_raw[:, ib:ib + 1], pT[:], axis=AX.X)

            bs = apool.tile([128, 4], f32, tag="bs", name="bs")
            nc.vector.tensor_mul(bs[:], bs_raw[:], bmask[:])
            bs16 = apool.tile([128, 4], f16, tag="bs16", name="bs16")
            nc.vector.tensor_copy(bs16[:], bs[:])
            nc.sync.dma_start(o_bs[:], bs[:])

            # ---- windows: [Z | E | bs] through banded matmul ----
            sc_row = apool.tile([1, 512], f32, tag="sc_row", name="sc_row")
            ss_row = apool.tile([1, 512], f32, tag="ss_row", name="ss_row")
            den = apool.tile([128, 4], f32, tag="den", name="den")
            rden = apool.tile([128, 4], f32, tag="rden", name="rden")
            r16 = apool.tile([128, 4], f16, tag="r16", name="r16")
            out_sb = apool.tile([1, 256], f32, tag="out_sb", name="out_sb")
            with tc.tile_pool(name="ps_w", bufs=4, space="PSUM") as pw_pool, \
                 tc.tile_pool(name="ps_t2", bufs=1, space="PSUM") as pst2, \
                 tc.tile_pool(name="ps_o", bufs=1, space="PSUM") as po_pool:
                zt = []
                for kt in range(4):
                    z = apool.tile([128, 258], f16, tag=f"zt{kt}", name="z")
                    nc.vector.tensor_scalar_mul(z[:, 0:256], bE[:, kt, :],
                                                E[:, kt:kt + 1])
                    nc.vector.tensor_copy(z[:, 256:257], E[:, kt:kt + 1])
                    nc.vector.tensor_copy(z[:, 257:258], bs16[:, kt:kt + 1])
                    zt.append(z)
                pw = []
                for mc in range(4):
                    p = pw_pool.tile([128, 258], f32, tag="pw", name="p")
                    for kt in range(4):
                        nc.tensor.matmul(p[:], BTw[:, kt, mc * 128:(mc + 1) * 128],
                                         zt[kt][:], start=(kt == 0), stop=(kt == 3))
                    pw.append(p)
                # win score row -> [1, 512]
                for mc in range(4):
                    wcol = apool.tile([128, 1], f32, tag="wcol", bufs=2, name="wcol")
                    nc.vector.tensor_copy(wcol[:], pw[mc][:, 257:258])
                    prT = pst2.tile([1, 128], f32, tag="prT", name="prT")
                    nc.tensor.transpose(prT[:], wcol[:], eye[:])
                    nc.vector.tensor_copy(sc_row[0:1, mc * 128:(mc + 1) * 128], prT[:])
                nc.vector.memset(sc_row[0:1, N_TOT:512], -1e9)
                # softmax over the 402 window scores
                mx = apool.tile([1, 1], f32, tag="mx", name="mx")
                nc.vector.reduce_max(mx[:], sc_row[:], axis=AX.X)
                nmx = apool.tile([1, 1], f32, tag="nmx", name="nmx")
                nc.vector.tensor_scalar_mul(nmx[:], mx[:], -1.0)
                es_row = apool.tile([1, 512], f32, tag="es_row", name="es_row")
                nc.scalar.activation(es_row[:], sc_row[:], AF.Exp, bias=nmx[0:1, 0:1])
                sm = apool.tile([1, 1], f32, tag="sm", name="sm")
                nc.vector.reduce_sum(sm[:], es_row[:], axis=AX.X)
                rsm = apool.tile([1, 1], f32, tag="rsm", name="rsm")
                nc.vector.reciprocal(rsm[:], sm[:])
                nc.vector.tensor_scalar_mul(ss_row[:], es_row[:], rsm[0:1, 0:1])
                nc.sync.dma_start(o_ss[:], ss_row[0:1, 0:N_TOT])
                # r = s_score / denom  (per window, as columns)
                for mc in range(4):
                    nc.vector.tensor_scalar_add(den[:, mc:mc + 1],
                                                pw[mc][:, 256:257], 1e-30)
                    nc.vector.reciprocal(rden[:, mc:mc + 1], den[:, mc:mc + 1])
                    pcol = pst2.tile([128, 1], f32, tag="pcol", name="pcol")
                    nc.tensor.transpose(pcol[:], ss_row[0:1, mc * 128:(mc + 1) * 128],
                                        eye[0:1, 0:1])
                    nc.vector.tensor_mul(r16[:, mc:mc + 1], pcol[:],
                                         rden[:, mc:mc + 1])
                # out = sum_n r[n] * seg_unnorm[n, :]
                pout = po_pool.tile([1, 256], f32, tag="pout", name="pout")
                for mc in range(4):
                    seg = apool.tile([128, 256], f16, tag="seg", bufs=2, name="seg")
                    nc.vector.tensor_copy(seg[:], pw[mc][:, 0:256])
                    nc.tensor.matmul(pout[:], r16[:, mc:mc + 1], seg[:],
                                     start=(mc == 0), stop=(mc == 3))
                nc.vector.tensor_copy(out_sb[:], pout[:])
                nc.sync.dma_start(o_out[:], out_sb[:])

    nc.compile()
    _cache["prog"] = nc
    return nc


def _prep_shared(inputs):
    """Host-side packing of replicated (weight) tensors."""
    f16 = np.float16
    f32 = np.float32
    sh = {}
    # Wb / Wc: [(l*3+m)*4+kt, 128, 512]; m order: 0=Wg, 1=Wn, 2=Wl
    for side, pfx in (("b", "bh"), ("c", "ch")):
        mats = [inputs[f"{pfx}_Wg"], inputs[f"{pfx}_Wn"], inputs[f"{pfx}_Wl"]]
        tiles = np.empty((24, 128, 512), f16)
        for l in range(2):
            for m in range(3):
                Wf = np.asarray(mats[m][l], f32).astype(f16)  # [512, 512]
                tiles[(l * 3 + m) * 4:(l * 3 + m) * 4 + 4] = Wf.reshape(4, 128, 512)
        sh[f"W{side}"] = tiles
        bias = np.empty((128, 24), f32)
        bmats = [inputs[f"{pfx}_bg"], inputs[f"{pfx}_bn"], inputs[f"{pfx}_bl"]]
        for l in range(2):
            for m in range(3):
                bf = np.asarray(bmats[m][l], f32)  # [512]
                bias[:, (l * 3 + m) * 4:(l * 3 + m) * 4 + 4] = bf.reshape(4, 128).T
        sh[f"bias{side}"] = bias
    sh["mWq"] = np.asarray(inputs["m_Wq"], f32).astype(f16).reshape(4, 128, 512)
    sh["mWk"] = np.asarray(inputs["m_Wk"], f32).astype(f16).reshape(4, 128, 512)
    sh["mbq"] = np.ascontiguousarray(
        np.asarray(inputs["m_bq"], f32).reshape(4, 128).T)
    sh["aWq"] = np.asarray(inputs["a_Wq"], f32).astype(f16).reshape(2, 128, 256)
    sh["aWk"] = np.asarray(inputs["a_Wk"], f32).astype(f16).reshape(2, 128, 256)
    sh["abq"] = np.ascontiguousarray(
        np.asarray(inputs["a_bq"], f32).reshape(2, 128).T)
    sh["va"] = np.ascontiguousarray(
        np.asarray(inputs["a_v"], f32).astype(f16).reshape(2, 128).T)
    vz = np.zeros((128, 1020), f16)
    mv = np.asarray(inputs["m_v"], f32).astype(f16)
    for c in range(4):
        vz[:, c * 255 + 127] = mv[c * 128:(c + 1) * 128]
    sh["vz"] = vz
    sh["BT"] = np.ascontiguousarray(
        _build_window_matrix_T().reshape(4, 128, 512))
    sh["eye"] = np.eye(128, dtype=f32)
    return sh


def kernel(**inputs):
    nc = _build_program()
    from concourse import bass_utils

    f16 = np.float16
    f32 = np.float32
    sh = _prep_shared(inputs)

    b_enc = np.asarray(inputs["b_enc_output"], f32)
    c_enc = np.asarray(inputs["c_enc_output"], f32)
    c_state = np.asarray(inputs["c_state"], f32)
    b_mask = np.asarray(inputs["b_mask"]).astype(f32)
    c_mask = np.asarray(inputs["c_mask"]).astype(f32)

    in_maps = []
    for b in range(N_CORES):
        be = b_enc[b].astype(f16)                      # [512, 256]
        beT = np.ascontiguousarray(be.T)               # [256, 512]
        ce = c_enc[b].astype(f16)
        csv = c_state[b].astype(f16)                   # [1, 256]
        csTv = np.ascontiguousarray(csv.T)             # [256, 1]
        m = {
            "bT": beT.reshape(2, 128, 512),
            "bE": be.reshape(4, 128, 256),
            "cT": np.ascontiguousarray(ce.T).reshape(2, 128, 128),
            "csb": np.ascontiguousarray(
                np.broadcast_to(csTv, (256, 512))).reshape(2, 128, 512),
            "csT": csTv.reshape(2, 128, 1),
            "Wb": sh["Wb"], "Wc": sh["Wc"],
            "biasb": sh["biasb"], "biasc": sh["biasc"],
            "mWq": sh["mWq"], "mWk": sh["mWk"], "mbq": sh["mbq"],
            "aWq": sh["aWq"], "aWk": sh["aWk"], "abq": sh["abq"],
            "va": sh["va"], "vz": sh["vz"], "BT": sh["BT"], "eye": sh["eye"],
            "cmadd": ((c_mask[b] - 1.0) * 1e9).reshape(128, 1).astype(f32),
            "bmask": np.ascontiguousarray(
                b_mask[b].reshape(4, 128).T).astype(f32),
        }
        in_maps.append(m)

    res = bass_utils.run_bass_kernel_spmd(nc, in_maps,
                                          core_ids=list(range(N_CORES)))
    out = np.stack([res.results[c]["o_out"] for c in range(N_CORES)])  # [8,1,256]
    ss = np.stack([res.results[c]["o_ss"][0] for c in range(N_CORES)])  # [8,402]
    bs = np.stack([np.ascontiguousarray(res.results[c]["o_bs"].T).reshape(512)
                   for c in range(N_CORES)])                            # [8,512]
    return (out.astype(f32), ss.astype(f32), bs.astype(f32))
